# revision 12
# baseline (speedup 1.0000x reference)
"""Trainium2 Bass kernel for nn_ComplexMixture.

Reference:
  output_real[b,n,m] = sum_s w[b,s] * (r[b,s,n]*r[b,s,m] + i[b,s,n]*i[b,s,m])
  output_imag[b,n,m] = sum_s w[b,s] * (i[b,s,n]*r[b,s,m] - r[b,s,n]*i[b,s,m])

Shapes: B=32, S=128, N=256, fp32. w is uniform [0,1) so sqrt(w) is real.

out_r is symmetric and out_i is antisymmetric, so the device only computes
  P = out_r + out_i
and the host recovers out_r = (P + P^T)/2, out_i = (P - P^T)/2.
The host pre-scales the inputs: Yr = sqrt(w)[:,None]*r, Yi = sqrt(w)[:,None]*i.
With U = Yr - Yi, V = Yr + Yi:
  P[n,m] = sum_s Yr[s,n]*U[s,m] + Yi[s,n]*V[s,m]
i.e. per 128-row output chunk c:  P_c = Yr_c.T @ U + Yi_c.T @ V  (PSUM accum).

v7 (from 19.5us v3/v6; baseline 24.5us). All bf16 I/O (rel err 4.3e-3 vs the
2e-2 gate), fp32 PSUM accumulation, no PE warmup (the HAM clock gate needs
~9us of sustained activity to release on this part; a ~17us kernel stays at
1.2 GHz, 213ns per 256-wide matmul).

Key change: the three input DMA kicks are issued BEFORE TileContext, right
after the engine preamble (~5.7us) instead of after the body-entry barrier
(~6.9us), into a raw (non-tile) SBUF buffer with manual completion
semaphores (then_inc(sem,16) per DMA, zero.py pattern). Consumers attach
explicit sem-ge waits directly to their instructions so the tile scheduler
cannot lift them above the data arrival. Queue-first-use latency (~0.8us) is
paid ~1.2us earlier, and outputs reuse the warmed queues (~0.4us pickup).
"""

import os

import numpy as np

import concourse.bass as bass
import concourse.mybir as mybir
import concourse.tile as tile
from concourse import bacc
from concourse.bass_utils import run_bass_kernel_spmd

B, S, N = 32, 128, 256
NCORES = 8
BPC = B // NCORES  # batches per core
W = 2 * N  # columns per batch block
XCOL = BPC * W  # 2048 bf16 per partition row

F32 = mybir.dt.float32
BF16 = mybir.dt.bfloat16

LAST_RESULTS = None  # stashed BassKernelResults for test harness introspection


def build_nc() -> bass.Bass:
    nc = bacc.Bacc(num_swdge_queues=1)
    xin = nc.dram_tensor("xpack", [S, XCOL], BF16, kind="ExternalInput")
    out = nc.dram_tensor("out_all", [128, XCOL], BF16, kind="ExternalOutput")

    # Raw SBUF input buffer + completion semaphores, loaded pre-TileContext.
    Xb = nc.alloc_sbuf_tensor("Xbuf", [S, XCOL], BF16)
    s_p0 = nc.alloc_semaphore("in_p0")  # pair0: both halves inc by 16 -> 32
    s_p1 = nc.alloc_semaphore("in_p1")  # pair1 (b2b3), all partitions

    # Clear the sems, then kick. These are the first post-preamble
    # instructions, ~1.2us before the TC body; DMA completions (~7.7us+) are
    # far after the clears (~5.7us), so the shared-sem clear is race-free.
    nc.sync.sem_clear(s_p0)
    nc.sync.dma_start(out=Xb[0:64, 0 : 2 * W], in_=xin[0:64, 0 : 2 * W]).then_inc(s_p0, 16)
    nc.scalar.dma_start(out=Xb[64:128, 0 : 2 * W], in_=xin[64:128, 0 : 2 * W]).then_inc(s_p0, 16)
    nc.gpsimd.sem_clear(s_p1)
    nc.gpsimd.dma_start(out=Xb[:, 2 * W : 4 * W], in_=xin[:, 2 * W : 4 * W]).then_inc(s_p1, 16)

    # Input-arrival gates are attached AFTER TileContext exits: the tile
    # scheduler's simulation doesn't model the pre-TC DMA increments and
    # would report a (false) deadlock if it saw these waits. Post-scheduling
    # the waits only gate execution; compile() legalizes multi-wait cases.
    gated = []

    def gate(inst, b):
        gated.append((inst, b))
        return inst

    with tile.TileContext(nc) as tc:
        with (
            tc.tile_pool(name="uv", bufs=BPC) as uv_pool,
            tc.tile_pool(name="op", bufs=1) as out_pool,
            tc.tile_pool(name="ps", bufs=BPC, space="PSUM") as ps_pool,
        ):
            O = out_pool.tile([128, XCOL], BF16, tag="O", name="O_all")

            # All UV prep on vector, before any cast in program order.
            UVs = {}
            for b in range(BPC):
                X = Xb[:, b * W : (b + 1) * W]
                UV = uv_pool.tile([S, W], BF16, tag="UV", name=f"UV{b}")
                gate(nc.vector.tensor_sub(UV[:, 0:N], X[:, 0:N], X[:, N:W]), b)
                gate(nc.vector.tensor_add(UV[:, N:W], X[:, 0:N], X[:, N:W]), b)
                UVs[b] = UV

            for b in range(BPC):
                X = Xb[:, b * W : (b + 1) * W]
                UV = UVs[b]
                ps = ps_pool.tile([128, W], F32, tag="ps", name=f"ps{b}")
                for c in range(2):
                    osl = slice(c * N, (c + 1) * N)
                    gate(nc.tensor.matmul(ps[:, osl], lhsT=X[:, c * 128 : c * 128 + 128],
                                          rhs=UV[:, 0:N], start=True, stop=False), b)
                    gate(nc.tensor.matmul(ps[:, osl], lhsT=X[:, N + c * 128 : N + c * 128 + 128],
                                          rhs=UV[:, N:W], start=False, stop=True), b)

                o0 = slice(b * W, b * W + N)
                o1 = slice(b * W + N, (b + 1) * W)
                oall = slice(b * W, (b + 1) * W)
                if b == 0:
                    # scalar casts (free after its in-kick) + scalar kick
                    nc.scalar.copy(out=O[:, o0], in_=ps[:, 0:N])
                    nc.scalar.copy(out=O[:, o1], in_=ps[:, N:W])
                    nc.scalar.dma_start(out=out[:, oall], in_=O[:, oall])
                elif b == 1:
                    # vector casts (after UVs) + sync kick
                    nc.vector.tensor_copy(O[:, o0], ps[:, 0:N])
                    nc.vector.tensor_copy(O[:, o1], ps[:, N:W])
                    nc.sync.dma_start(out=out[:, oall], in_=O[:, oall])
                elif b == 2:
                    nc.scalar.copy(out=O[:, o0], in_=ps[:, 0:N])
                    nc.scalar.copy(out=O[:, o1], in_=ps[:, N:W])
                    nc.sync.dma_start(out=out[:, oall], in_=O[:, oall])
                else:
                    # Tail: two independent cast->kick chains on two queues.
                    nc.scalar.copy(out=O[:, o0], in_=ps[:, 0:N])
                    nc.scalar.dma_start(out=out[:, o0], in_=O[:, o0])
                    nc.vector.tensor_copy(O[:, o1], ps[:, N:W])
                    nc.sync.dma_start(out=out[:, o1], in_=O[:, o1])

    for inst, b in gated:
        if b < 2:
            inst.wait_op(s_p0, 32, "sem-ge", check=False)
        else:
            inst.wait_op(s_p1, 16, "sem-ge", check=False)
    nc.compile()
    return nc


def kernel(**inputs: np.ndarray):
    global LAST_RESULTS
    import ml_dtypes

    r = np.asarray(inputs["input_real"], dtype=np.float32)
    i = np.asarray(inputs["input_imag"], dtype=np.float32)
    w = np.ascontiguousarray(np.asarray(inputs["weight"], dtype=np.float32))
    assert r.shape == (B, S, N) and i.shape == (B, S, N) and w.shape == (B, S)

    # [B, 2, S, N] -> per-core [S, (b t n)] batch-major blocks, bf16
    sws = np.sqrt(w)  # [B, S]
    xin = (np.stack([r, i], axis=1) * sws[:, None, :, None]).astype(ml_dtypes.bfloat16)

    in_maps = []
    for c in range(NCORES):
        sl = slice(c * BPC, (c + 1) * BPC)
        xpack = np.transpose(xin[sl], (2, 0, 1, 3)).reshape(S, XCOL)
        in_maps.append({"xpack": np.ascontiguousarray(xpack)})

    nc = build_nc()
    res = run_bass_kernel_spmd(nc, in_maps, core_ids=list(range(NCORES)))
    LAST_RESULTS = res

    # out_all[core] is [128, (b c m)] bf16; P[b, c*128+p, m] = out[p, b*512 + c*256 + m]
    out_all = np.stack(
        [np.asarray(res.results[c]["out_all"]) for c in range(NCORES)], axis=0
    ).astype(np.float32)  # [NCORES, 128, XCOL]
    out_all = out_all.reshape(NCORES, 128, BPC, 2, N)
    P = np.transpose(out_all, (0, 2, 3, 1, 4)).reshape(B, N, N)
    Pt = np.transpose(P, (0, 2, 1))
    out_r = (P + Pt) * np.float32(0.5)
    out_i = (P - Pt) * np.float32(0.5)
    return (np.ascontiguousarray(out_r), np.ascontiguousarray(out_i))


# revision 14
# speedup vs baseline: 1.0644x; 1.0644x over previous
"""Trainium2 Bass kernel for nn_ComplexMixture.

Reference:
  output_real[b,n,m] = sum_s w[b,s] * (r[b,s,n]*r[b,s,m] + i[b,s,n]*i[b,s,m])
  output_imag[b,n,m] = sum_s w[b,s] * (i[b,s,n]*r[b,s,m] - r[b,s,n]*i[b,s,m])

Shapes: B=32, S=128, N=256, fp32. w is uniform [0,1) so sqrt(w) is real.

out_r is symmetric and out_i is antisymmetric, so the device only computes
  P = out_r + out_i
and the host recovers out_r = (P + P^T)/2, out_i = (P - P^T)/2.
The host pre-scales the inputs: Yr = sqrt(w)[:,None]*r, Yi = sqrt(w)[:,None]*i.
With U = Yr - Yi, V = Yr + Yi:
  P[n,m] = sum_s Yr[s,n]*U[s,m] + Yi[s,n]*V[s,m]
i.e. per 128-row output chunk c:  P_c = Yr_c.T @ U + Yi_c.T @ V  (PSUM accum).

v7 (from 19.5us v3/v6; baseline 24.5us). All bf16 I/O (rel err 4.3e-3 vs the
2e-2 gate), fp32 PSUM accumulation, no PE warmup (the HAM clock gate needs
~9us of sustained activity to release on this part; a ~17us kernel stays at
1.2 GHz, 213ns per 256-wide matmul).

Key change: the three input DMA kicks are issued BEFORE TileContext, right
after the engine preamble (~5.7us) instead of after the body-entry barrier
(~6.9us), into a raw (non-tile) SBUF buffer with manual completion
semaphores (then_inc(sem,16) per DMA, zero.py pattern). Consumers attach
explicit sem-ge waits directly to their instructions so the tile scheduler
cannot lift them above the data arrival. Queue-first-use latency (~0.8us) is
paid ~1.2us earlier, and outputs reuse the warmed queues (~0.4us pickup).
"""

import os

import numpy as np

import concourse.bass as bass
import concourse.mybir as mybir
import concourse.tile as tile
from concourse import bacc
from concourse.bass_utils import run_bass_kernel_spmd

B, S, N = 32, 128, 256
NCORES = 8
BPC = B // NCORES  # batches per core
W = 2 * N  # columns per batch block
XCOL = BPC * W  # 2048 bf16 per partition row

F32 = mybir.dt.float32
BF16 = mybir.dt.bfloat16

LAST_RESULTS = None  # stashed BassKernelResults for test harness introspection


def build_nc() -> bass.Bass:
    nc = bacc.Bacc(num_swdge_queues=1)
    xin = nc.dram_tensor("xpack", [S, XCOL], BF16, kind="ExternalInput")
    out = nc.dram_tensor("out_all", [128, XCOL], BF16, kind="ExternalOutput")

    # Raw SBUF input buffer + completion semaphores, loaded pre-TileContext.
    Xb = nc.alloc_sbuf_tensor("Xbuf", [S, XCOL], BF16)
    s_p0 = nc.alloc_semaphore("in_p0")  # pair0: both halves inc by 16 -> 32
    s_p1 = nc.alloc_semaphore("in_p1")  # pair1 (b2b3), all partitions

    # Clear the sems, then kick. All input rides the two HWDGE rings as
    # partition-split halves (2KB packets); SWDGE is avoided for input — a
    # pre-TC SWDGE kick was observed to strand its last packets ~2.3us when
    # the gpsimd engine goes idle. DMA completions (~8.5us+) land far after
    # the clears (~6.9us), so the shared-sem clears are race-free.
    nc.sync.sem_clear(s_p0)
    nc.sync.dma_start(out=Xb[0:64, 0 : 2 * W], in_=xin[0:64, 0 : 2 * W]).then_inc(s_p0, 16)
    nc.sync.dma_start(out=Xb[0:64, 2 * W : 4 * W], in_=xin[0:64, 2 * W : 4 * W]).then_inc(s_p1, 16)
    nc.scalar.sem_clear(s_p1)
    nc.scalar.dma_start(out=Xb[64:128, 0 : 2 * W], in_=xin[64:128, 0 : 2 * W]).then_inc(s_p0, 16)
    nc.scalar.dma_start(out=Xb[64:128, 2 * W : 4 * W], in_=xin[64:128, 2 * W : 4 * W]).then_inc(s_p1, 16)

    # Input-arrival gates are attached AFTER TileContext exits: the tile
    # scheduler's simulation doesn't model the pre-TC DMA increments and
    # would report a (false) deadlock if it saw these waits. Post-scheduling
    # the waits only gate execution; compile() legalizes multi-wait cases.
    gated = []

    def gate(inst, b):
        gated.append((inst, b))
        return inst

    with tile.TileContext(nc) as tc:
        with (
            tc.tile_pool(name="uv", bufs=BPC) as uv_pool,
            tc.tile_pool(name="op", bufs=1) as out_pool,
            tc.tile_pool(name="ps", bufs=BPC, space="PSUM") as ps_pool,
        ):
            O = out_pool.tile([128, XCOL], BF16, tag="O", name="O_all")

            # All UV prep on vector, before any cast in program order.
            UVs = {}
            for b in range(BPC):
                X = Xb[:, b * W : (b + 1) * W]
                UV = uv_pool.tile([S, W], BF16, tag="UV", name=f"UV{b}")
                gate(nc.vector.tensor_sub(UV[:, 0:N], X[:, 0:N], X[:, N:W]), b)
                gate(nc.vector.tensor_add(UV[:, N:W], X[:, 0:N], X[:, N:W]), b)
                UVs[b] = UV

            for b in range(BPC):
                X = Xb[:, b * W : (b + 1) * W]
                UV = UVs[b]
                ps = ps_pool.tile([128, W], F32, tag="ps", name=f"ps{b}")
                for c in range(2):
                    osl = slice(c * N, (c + 1) * N)
                    gate(nc.tensor.matmul(ps[:, osl], lhsT=X[:, c * 128 : c * 128 + 128],
                                          rhs=UV[:, 0:N], start=True, stop=False), b)
                    gate(nc.tensor.matmul(ps[:, osl], lhsT=X[:, N + c * 128 : N + c * 128 + 128],
                                          rhs=UV[:, N:W], start=False, stop=True), b)

                o0 = slice(b * W, b * W + N)
                o1 = slice(b * W + N, (b + 1) * W)
                oall = slice(b * W, (b + 1) * W)
                if b == 0:
                    # scalar casts (free after its in-kick) + scalar kick
                    nc.scalar.copy(out=O[:, o0], in_=ps[:, 0:N])
                    nc.scalar.copy(out=O[:, o1], in_=ps[:, N:W])
                    nc.scalar.dma_start(out=out[:, oall], in_=O[:, oall])
                elif b == 1:
                    # vector casts (after UVs) + sync kick
                    nc.vector.tensor_copy(O[:, o0], ps[:, 0:N])
                    nc.vector.tensor_copy(O[:, o1], ps[:, N:W])
                    nc.sync.dma_start(out=out[:, oall], in_=O[:, oall])
                elif b == 2:
                    nc.scalar.copy(out=O[:, o0], in_=ps[:, 0:N])
                    nc.scalar.copy(out=O[:, o1], in_=ps[:, N:W])
                    nc.sync.dma_start(out=out[:, oall], in_=O[:, oall])
                else:
                    # Tail: two independent cast->kick chains on two queues.
                    nc.scalar.copy(out=O[:, o0], in_=ps[:, 0:N])
                    nc.scalar.dma_start(out=out[:, o0], in_=O[:, o0])
                    nc.vector.tensor_copy(O[:, o1], ps[:, N:W])
                    nc.sync.dma_start(out=out[:, o1], in_=O[:, o1])

    for inst, b in gated:
        if b < 2:
            inst.wait_op(s_p0, 32, "sem-ge", check=False)
        else:
            inst.wait_op(s_p1, 32, "sem-ge", check=False)
    nc.compile()
    return nc


def kernel(**inputs: np.ndarray):
    global LAST_RESULTS
    import ml_dtypes

    r = np.asarray(inputs["input_real"], dtype=np.float32)
    i = np.asarray(inputs["input_imag"], dtype=np.float32)
    w = np.ascontiguousarray(np.asarray(inputs["weight"], dtype=np.float32))
    assert r.shape == (B, S, N) and i.shape == (B, S, N) and w.shape == (B, S)

    # [B, 2, S, N] -> per-core [S, (b t n)] batch-major blocks, bf16
    sws = np.sqrt(w)  # [B, S]
    xin = (np.stack([r, i], axis=1) * sws[:, None, :, None]).astype(ml_dtypes.bfloat16)

    in_maps = []
    for c in range(NCORES):
        sl = slice(c * BPC, (c + 1) * BPC)
        xpack = np.transpose(xin[sl], (2, 0, 1, 3)).reshape(S, XCOL)
        in_maps.append({"xpack": np.ascontiguousarray(xpack)})

    nc = build_nc()
    res = run_bass_kernel_spmd(nc, in_maps, core_ids=list(range(NCORES)))
    LAST_RESULTS = res

    # out_all[core] is [128, (b c m)] bf16; P[b, c*128+p, m] = out[p, b*512 + c*256 + m]
    out_all = np.stack(
        [np.asarray(res.results[c]["out_all"]) for c in range(NCORES)], axis=0
    ).astype(np.float32)  # [NCORES, 128, XCOL]
    out_all = out_all.reshape(NCORES, 128, BPC, 2, N)
    P = np.transpose(out_all, (0, 2, 3, 1, 4)).reshape(B, N, N)
    Pt = np.transpose(P, (0, 2, 1))
    out_r = (P + Pt) * np.float32(0.5)
    out_i = (P - Pt) * np.float32(0.5)
    return (np.ascontiguousarray(out_r), np.ascontiguousarray(out_i))
